# revision 23
# baseline (speedup 1.0000x reference)
"""Trainium2 8-core kernel for batched attention + concat projection.

Reference computation (per batch b):
    scores = Q @ C^T                  [TQ, TC]
    A      = softmax(scores, axis=-1)
    mix    = A @ C                    [TQ, H]
    out    = tanh(concat([mix, Q]) @ W^T)   [TQ, H]

Distribution: pure data-parallel over batch (B=16 across 8 cores, 2
batches per core), W replicated. No collectives needed.

Per-core dataflow (all activations kept in "transposed" [feature, token]
layout so every matmul contracts over the partition axis):
  - CT = C^T, QT = Q^T built on-device via PE transposes (fp32r).
  - scores tile S[q,k] = QT.T @ CT  (float32r matmuls, full speed).
  - softmax over free axis k: DVE reduce_max(negate) -> ACT exp with
    per-partition bias + accumulated row sums -> DVE reciprocal ->
    DVE in-place scale (normalized P in bf16).
  - P^T via PE transposes (bf16), mix^T = C.T @ P^T (bf16 matmuls,
    lhsT = C in natural [k, h] layout).
  - proj: out[q, :] = tanh(combT.T @ W^T) where combT = [mix^T; Q^T]
    in bf16, W^T pre-transposed on host (weight layout choice).

The P^T/PV/proj stages for super-iteration s are emitted one
super-iteration later (software pipelining) so the in-order TensorE
stream always has ready matmul work while the softmax chain of the
current tile runs on ACT/DVE; this keeps the PE HAM clock-gate warm.
Transposes are packed 4 (f32) / 8 (bf16) per PSUM bank so each bank is
drained by a single wide ACT/DVE copy instead of many small ones.
"""

import numpy as np
import ml_dtypes

import concourse.bacc as bacc
import concourse.tile as tile
import concourse.mybir as mybir
from concourse.bass_utils import run_bass_kernel_spmd

F32 = mybir.dt.float32
F32R = mybir.dt.float32r
BF16 = mybir.dt.bfloat16

N_CORES = 8
B, TQ, TC, H = 16, 2048, 2048, 1024
PT_VIA_DMA = False


def build_bass(b_loc, tq, tc, h, n_cores=N_CORES):
    """Build the per-core Bass graph. All cores run the same graph (SPMD)."""
    d = 2 * h
    ho = h
    n_qt = tq // 128       # q tiles
    n_kt = tc // 128       # k tiles
    n_hc = h // 128        # h chunks
    n_dc = d // 128        # d chunks (contraction for proj)
    kb = min(512, tc)      # QK rhs block (fp32 moving-operand max)
    n_kb = tc // kb
    hob = min(512, ho)     # proj output block
    n_hob = ho // hob
    SUPER = 2              # q-tiles per super-iteration
    assert n_qt % SUPER == 0
    n_s = n_qt // SUPER
    sq = SUPER * 128       # q columns per super-iteration
    qg = min(4, n_hc)      # f32 transposes packed per PSUM bank
    pg = min(8, n_kt)      # bf16 transposes packed per PSUM bank

    nc = bacc.Bacc("TRN2", target_bir_lowering=False, debug=False,
                   num_devices=n_cores)

    q_ext = nc.declare_dram_parameter("q", [b_loc, tq, h], F32, isOutput=False)
    c_ext = nc.declare_dram_parameter("c", [b_loc, tc, h], F32, isOutput=False)
    wt_ext = nc.declare_dram_parameter("wt", [d, ho], F32, isOutput=False)
    idf_ext = nc.declare_dram_parameter("idf", [128, 128], F32, isOutput=False)
    idb_ext = nc.declare_dram_parameter("idb", [128, 128], BF16, isOutput=False)
    out_ext = nc.declare_dram_parameter("out", [b_loc, tq, ho], F32, isOutput=True)

    with tile.TileContext(nc) as tc_:
        with (
            tc_.tile_pool(name="const", bufs=1) as const_pool,
            tc_.tile_pool(name="stage", bufs=5) as stage_pool,
            tc_.tile_pool(name="ct", bufs=1) as ct_pool,
            tc_.tile_pool(name="cbf", bufs=1) as cbf_pool,
            tc_.tile_pool(name="qt", bufs=2) as qt_pool,
            tc_.tile_pool(name="p", bufs=2 * SUPER) as p_pool,
            tc_.tile_pool(name="ptb", bufs=1) as pt_pool,
            tc_.tile_pool(name="comb", bufs=2) as comb_pool,
            tc_.tile_pool(name="ostage", bufs=2) as out_pool,
            tc_.tile_pool(name="stats", bufs=12) as stats_pool,
            tc_.tile_pool(name="ps_s", bufs=1, space="PSUM") as ps_s,
            tc_.tile_pool(name="ps_tp", bufs=2, space="PSUM") as ps_tp,
            tc_.tile_pool(name="ps_mm", bufs=2, space="PSUM") as ps_mm,
        ):
            # --- constants: identities + W^T (bf16) ---
            idf = const_pool.tile([128, 128], F32, tag="idf")
            nc.sync.dma_start(idf[:], idf_ext[:])
            idb = const_pool.tile([128, 128], BF16, tag="idb")
            nc.sync.dma_start(idb[:], idb_ext[:])

            wt_bf = const_pool.tile([128, n_dc * ho], BF16, tag="wtbf")

            def emit_wt_setup():
                for dc in range(n_dc):
                    ws = stage_pool.tile([128, ho], F32, tag="stage",
                                         name=f"ws_{dc}")
                    nc.sync.dma_start(ws[:], wt_ext[dc * 128:(dc + 1) * 128, :])
                    if dc % 2 == 0:
                        nc.vector.tensor_copy(
                            wt_bf[:, dc * ho:(dc + 1) * ho], ws[:])
                    else:
                        nc.scalar.copy(wt_bf[:, dc * ho:(dc + 1) * ho], ws[:])

            p_tiles = {}      # (b, t) -> normalized P tile
            combT_map = {}    # s -> combT tile of current batch
            pt_map = {}       # s -> P^T tile of current batch

            def emit_qtr(b, s, ti, qs=None):
                """Q load + QT transposes; returns qt_t for the QK stage."""
                t = s * SUPER + ti
                combT = combT_map[(b, s)]
                comb_r = combT.rearrange("p (dc q) -> p dc q", q=sq)
                if qs is None:
                    qs = stage_pool.tile([128, h], F32, tag="stage",
                                         name=f"qs_{b}_{t}")
                    nc.sync.dma_start(qs[:], q_ext[b, t * 128:(t + 1) * 128, :])
                qt_t = qt_pool.tile([128, h], F32R, tag="qt",
                                    name=f"qt_{b}_{t}")
                for g in range(n_hc // qg):
                    tq4 = ps_tp.tile([128, qg * 128], F32, tag="tp",
                                     name=f"tq4_{b}_{t}_{g}")
                    for j in range(qg):
                        hc = qg * g + j
                        nc.tensor.transpose(
                            tq4[:, j * 128:(j + 1) * 128],
                            qs[:, hc * 128:(hc + 1) * 128], idf[:])
                    dst = qt_t[:, g * qg * 128:(g + 1) * qg * 128]
                    if g % 2 == 0:
                        nc.scalar.copy(dst, tq4[:])
                    else:
                        nc.vector.tensor_copy(dst, tq4[:])
                nc.vector.tensor_copy(
                    comb_r[:, n_hc: 2 * n_hc, ti * 128:(ti + 1) * 128],
                    qt_t.rearrange("p (j c) -> p j c", c=128)[:])
                return qt_t

            def emit_qk_softmax(b, s, ti, qt_t, ct_all):
                """QK matmuls into one PSUM tile + wide softmax."""
                t = s * SUPER + ti
                s_ps = ps_s.tile([128, tc], F32, tag="s", name=f"s_{b}_{t}")
                if s == 0:
                    loop = [(hc, kbi) for kbi in range(n_kb)
                            for hc in range(n_hc)]
                else:
                    loop = [(hc, kbi) for hc in range(n_hc)
                            for kbi in range(n_kb)]
                for hc, kbi in loop:
                    lhs = qt_t[:, hc * 128:(hc + 1) * 128]
                    rhs = ct_all[:, hc * tc + kbi * kb:
                                 hc * tc + (kbi + 1) * kb]
                    nc.tensor.matmul(
                        s_ps[:, kbi * kb:(kbi + 1) * kb], lhs, rhs,
                        start=(hc == 0), stop=(hc == n_hc - 1))

                negm = stats_pool.tile([128, 1], F32, tag="negm",
                                       name=f"negm_{b}_{t}")
                nc.vector.reduce_max(
                    negm[:], s_ps[:], axis=mybir.AxisListType.X, negate=True)
                l_tot = stats_pool.tile([128, 1], F32, tag="ltot",
                                        name=f"lt_{b}_{t}")
                nc.vector.memset(l_tot[:], 0.0)
                p = p_pool.tile([128, tc], BF16, tag="p", name=f"p_{b}_{t}")
                nc.scalar.activation(
                    p[:], s_ps[:], mybir.ActivationFunctionType.Exp,
                    bias=negm[:], scale=1.0, accum_out=l_tot[:])
                rcp = stats_pool.tile([128, 1], F32, tag="rcp",
                                      name=f"rcp_{b}_{t}")
                nc.vector.reciprocal(rcp[:], l_tot[:])
                nc.vector.tensor_scalar_mul(p[:], p[:], rcp[:])
                p_tiles[(b, t)] = p

            def emit_pt(b, s):
                """P^T for super s (consumes p tiles). Either xbar DMA
                transposes (keeps TensorE free) or PE transposes packed
                into PSUM banks drained by wide ACT copies."""
                pt_big = pt_pool.tile([128, n_kt * sq], BF16, tag="ptb",
                                      name=f"ptb_{b}_{s}")
                pt_r = pt_big.rearrange("p (k q) -> p k q", q=sq)
                for ti in range(SUPER):
                    p = p_tiles.pop((b, s * SUPER + ti))
                    if PT_VIA_DMA:
                        for kt in range(n_kt):
                            nc.sync.dma_start_transpose(
                                out=pt_big[:, kt * sq + ti * 128:
                                           kt * sq + (ti + 1) * 128],
                                in_=p[:, kt * 128:(kt + 1) * 128])
                        continue
                    for g in range(n_kt // pg):
                        tp8 = ps_tp.tile([128, pg * 128], BF16, tag="tp",
                                         name=f"tp8_{b}_{s}_{ti}_{g}")
                        for j in range(pg):
                            kt = pg * g + j
                            nc.tensor.transpose(
                                tp8[:, j * 128:(j + 1) * 128],
                                p[:, kt * 128:(kt + 1) * 128], idb[:])
                        nc.scalar.copy(
                            pt_r[:, pg * g: pg * (g + 1),
                                 ti * 128:(ti + 1) * 128],
                            tp8.rearrange("p (j c) -> p j c", c=128)[:])
                pt_map[(b, s)] = pt_big

            def emit_pv(b, s, c_bf):
                """PV matmuls: mix^T chunks into combT for super s."""
                combT = combT_map[(b, s)]
                pt_big = pt_map.pop((b, s))
                for hc in range(n_hc):
                    mm = ps_mm.tile([128, sq], F32, tag="mm",
                                    name=f"mm_{b}_{s}_{hc}")
                    for kt in range(n_kt):
                        nc.tensor.matmul(
                            mm[:],
                            c_bf[:, kt * h + hc * 128: kt * h + (hc + 1) * 128],
                            pt_big[:, kt * sq:(kt + 1) * sq],
                            start=(kt == 0), stop=(kt == n_kt - 1))
                    nc.vector.tensor_copy(
                        combT[:, hc * sq:(hc + 1) * sq], mm[:])

            def emit_proj(b, s):
                """Projection + tanh + store for both tiles of super s."""
                combT = combT_map.pop((b, s))
                for ti in range(SUPER):
                    t = s * SUPER + ti
                    ostage = out_pool.tile([128, ho], F32, tag="ostage",
                                           name=f"os_{b}_{t}")
                    for hb in range(n_hob):
                        pr = ps_mm.tile([128, hob], F32, tag="mm",
                                        name=f"pr_{b}_{t}_{hb}")
                        for dc in range(n_dc):
                            nc.tensor.matmul(
                                pr[:],
                                combT[:, dc * sq + ti * 128:
                                      dc * sq + (ti + 1) * 128],
                                wt_bf[:, dc * ho + hb * hob:
                                      dc * ho + (hb + 1) * hob],
                                start=(dc == 0), stop=(dc == n_dc - 1))
                        nc.scalar.activation(
                            ostage[:, hb * hob:(hb + 1) * hob], pr[:],
                            mybir.ActivationFunctionType.Tanh)
                    nc.sync.dma_start(
                        out_ext[b, t * 128:(t + 1) * 128, :], ostage[:])

            for b in range(b_loc):
                # prefetch the first super's Q tiles ahead of the C DMAs so
                # the first Qtr transposes are not stuck behind 16 MB of C/W
                q_pre = []
                for ti in range(SUPER):
                    qp = stage_pool.tile([128, h], F32, tag="stage",
                                         name=f"qpre_{b}_{ti}")
                    nc.sync.dma_start(qp[:], q_ext[b, ti * 128:(ti + 1) * 128, :])
                    q_pre.append(qp)
                # --- batch setup: CT (f32r, [h, k]) and C (bf16, [k, h]) ---
                ct_all = ct_pool.tile([128, n_hc * tc], F32R, tag="ct",
                                      name=f"ct_{b}")
                ct_r = ct_all.rearrange("p (hc k) -> p hc k", k=tc)
                c_bf = cbf_pool.tile([128, n_kt * h], BF16, tag="cbf",
                                     name=f"cbf_{b}")

                def emit_c_setup(kt):
                    cs = stage_pool.tile([128, h], F32, tag="stage",
                                         name=f"cs_{b}_{kt}")
                    nc.sync.dma_start(cs[:], c_ext[b, kt * 128:(kt + 1) * 128, :])
                    if kt % 2 == 0:
                        nc.vector.tensor_copy(
                            c_bf[:, kt * h:(kt + 1) * h], cs[:])
                    else:
                        nc.scalar.copy(c_bf[:, kt * h:(kt + 1) * h], cs[:])
                    for g in range(n_hc // qg):
                        tc4 = ps_tp.tile([128, qg * 128], F32, tag="tp",
                                         name=f"tc4_{b}_{kt}_{g}")
                        for j in range(qg):
                            hc = qg * g + j
                            nc.tensor.transpose(
                                tc4[:, j * 128:(j + 1) * 128],
                                cs[:, hc * 128:(hc + 1) * 128], idf[:])
                        dst = ct_r[:, qg * g: qg * (g + 1),
                                   kt * 128:(kt + 1) * 128]
                        src = tc4.rearrange("p (j c) -> p j c", c=128)[:]
                        if (g + kt) % 2 == 1:
                            nc.scalar.copy(dst, src)
                        else:
                            nc.vector.tensor_copy(dst, src)

                # first half of C, then the first Q-transpose (fills the
                # DMA-paced window), then the rest of C
                for kt in range(n_kt // 2):
                    emit_c_setup(kt)
                combT_map[(b, 0)] = comb_pool.tile(
                    [128, n_dc * sq], BF16, tag="comb", name=f"cb_{b}_0")
                qt0_first = emit_qtr(b, 0, 0, qs=q_pre[0])
                for kt in range(n_kt // 2, n_kt):
                    emit_c_setup(kt)
                if b == 0:
                    emit_wt_setup()

                # --- pipelined main loop ---
                for s in range(n_s):
                    if s > 0:
                        combT_map[(b, s)] = comb_pool.tile(
                            [128, n_dc * sq], BF16, tag="comb",
                            name=f"cb_{b}_{s}")
                    qt0 = qt0_first if s == 0 else emit_qtr(b, s, 0)
                    if s > 0:
                        emit_pt(b, s - 1)
                    emit_qk_softmax(b, s, 0, qt0, ct_all)
                    qt1 = emit_qtr(b, s, 1, qs=q_pre[1] if s == 0 else None)
                    if s > 0:
                        emit_pv(b, s - 1, c_bf)
                    emit_qk_softmax(b, s, 1, qt1, ct_all)
                    if s > 0:
                        emit_proj(b, s - 1)
                emit_pt(b, n_s - 1)
                emit_pv(b, n_s - 1, c_bf)
                emit_proj(b, n_s - 1)

    nc.compile()
    return nc


_NC_CACHE = {}


def _get_nc(b_loc, tq, tc, h):
    key = (b_loc, tq, tc, h)
    if key not in _NC_CACHE:
        _NC_CACHE[key] = build_bass(b_loc, tq, tc, h)
    return _NC_CACHE[key]


def make_in_maps(query, context, W_attn, n_cores=N_CORES):
    b = query.shape[0]
    b_loc = b // n_cores
    wt = np.ascontiguousarray(W_attn.T.astype(np.float32))
    idf = np.eye(128, dtype=np.float32)
    idb = np.eye(128).astype(ml_dtypes.bfloat16)
    in_maps = []
    for i in range(n_cores):
        in_maps.append({
            "q": np.ascontiguousarray(
                query[i * b_loc:(i + 1) * b_loc].astype(np.float32)),
            "c": np.ascontiguousarray(
                context[i * b_loc:(i + 1) * b_loc].astype(np.float32)),
            "wt": wt,
            "idf": idf,
            "idb": idb,
        })
    return in_maps


def kernel(query, context, W_attn, _trace=False, _trace_kwargs=None):
    b, tq, h = query.shape
    tc = context.shape[1]
    b_loc = b // N_CORES
    nc = _get_nc(b_loc, tq, tc, h)
    in_maps = make_in_maps(query, context, W_attn)
    res = run_bass_kernel_spmd(
        nc, in_maps, core_ids=list(range(N_CORES)), trace=_trace,
        **(_trace_kwargs or {}))
    out = np.concatenate([res.results[i]["out"] for i in range(N_CORES)], axis=0)
    if _trace:
        return out, res
    return out
